# revision 2
# baseline (speedup 1.0000x reference)
# Lagrangian-NN qddot kernel for TRN2 (8 NeuronCores, data-parallel over batch).
#
# Math: scalar L(q,qdot) = MLP(24->256x4->1, softplus). Per sample:
#   M = d2L/dqdot2 + 0.01 I ; C = d2L/dqdot dq ; qddot = M^-1 (dL/dq - C qdot).
# Batched fwd+bwd gives grad; 12 qdot-direction forward-over-reverse tangents give
# H[:,12:] whose symmetry supplies both M and the Coriolis contraction; per-sample
# 12x12 solve by batched no-pivot Gauss-Jordan (M ~ 0.01*I, cond ~1.06).
# softplus/sigmoid composed from {abs,exp,ln,relu} (single ACT table set):
#   Z = relu(x) + ln(exp(-|x|)+1) ; S = exp(x - Z).
import sys
import numpy as np

for p in ("/opt/trn_rl_repo", "/root/.axon_site/_ro/trn_rl_repo"):
    if p not in sys.path:
        sys.path.insert(0, p)

import concourse.bass as bass
import concourse.mybir as mybir
import concourse.tile as tile
from concourse import bacc
from concourse.bass_utils import run_bass_kernel_spmd

F32 = mybir.dt.float32
F32R = mybir.dt.float32r
AF = mybir.ActivationFunctionType
ALU = mybir.AluOpType

B, ND, H, NC = 8192, 12, 256, 8
N = B // NC
IN = 2 * ND
T = 64
NT = N // T
NG = N // 128
FD = ND * T
CH = 512
KT = H // 128

_cache = {}


def build_kernel():
    nc = bacc.Bacc("TRN2", target_bir_lowering=False)
    dq = nc.dram_tensor("q", (N, ND), F32, kind="ExternalInput")
    dqd = nc.dram_tensor("qdot", (N, ND), F32, kind="ExternalInput")
    dWT = [nc.dram_tensor(f"WT{l}", s, F32, kind="ExternalInput")
           for l, s in enumerate([(IN, H), (H, H), (H, H), (H, H)])]
    dWn = {l: nc.dram_tensor(f"Wn{l}", (H, H), F32, kind="ExternalInput") for l in (1, 2, 3)}
    dW0 = nc.dram_tensor("W0n", (H, IN), F32, kind="ExternalInput")
    dbs = [nc.dram_tensor(f"b{l}", (H, 1), F32, kind="ExternalInput") for l in range(4)]
    dw4 = nc.dram_tensor("w4", (H, 1), F32, kind="ExternalInput")
    dide = nc.dram_tensor("ident", (128, 128), F32, kind="ExternalInput")
    dout = nc.dram_tensor("qdd", (N, ND), F32, kind="ExternalOutput")

    with tile.TileContext(nc) as tc:
        with tc.tile_pool(name="wp", bufs=1) as wp, \
             tc.tile_pool(name="acts", bufs=1) as actp, \
             tc.tile_pool(name="tang", bufs=1) as tgp, \
             tc.tile_pool(name="scr", bufs=1) as scr, \
             tc.tile_pool(name="psB", bufs=2, space="PSUM") as psB, \
             tc.tile_pool(name="psT", bufs=2, space="PSUM") as psT:

            ident = wp.tile([128, 128], F32)
            nc.sync.dma_start(ident[:], dide[:])

            def load_round(dram, P, Fr, tag):
                parts = []
                for ki, p0 in enumerate(range(0, P, 128)):
                    pe = min(P, p0 + 128)
                    raw = scr.tile([pe - p0, Fr], F32, tag="wraw")
                    nc.sync.dma_start(raw[:], dram[p0:pe, :])
                    r = wp.tile([pe - p0, Fr], F32R, tag=f"{tag}_{ki}")
                    nc.scalar.activation(r[:], raw[:], AF.Copy)
                    parts.append(r)
                return parts

            WT_r = [load_round(dWT[l], (IN if l == 0 else H), H, f"WT{l}") for l in range(4)]
            Wn_r = {l: load_round(dWn[l], H, H, f"Wn{l}") for l in (1, 2, 3)}
            W0_r = load_round(dW0, H, IN, "W0n")
            W0q = []
            for ki in range(KT):
                t = wp.tile([128, ND], F32, tag=f"W0q{ki}")
                nc.sync.dma_start(t[:], dW0[ki * 128:(ki + 1) * 128, ND:])
                W0q.append(t)
            bs = []
            for l in range(4):
                ps_ = []
                for ki in range(KT):
                    t = wp.tile([128, 1], F32, tag=f"b{l}_{ki}")
                    nc.sync.dma_start(t[:], dbs[l][ki * 128:(ki + 1) * 128, :])
                    ps_.append(t)
                bs.append(ps_)
            w4t = []
            for ki in range(KT):
                t = wp.tile([128, 1], F32, tag=f"w4_{ki}")
                nc.sync.dma_start(t[:], dw4[ki * 128:(ki + 1) * 128, :])
                w4t.append(t)

            xts = []
            XT = actp.tile([IN, N], F32R)
            for g in range(NG):
                xt = actp.tile([128, IN], F32, tag=f"xt{g}")
                nc.sync.dma_start(xt[:, 0:ND], dq[g * 128:(g + 1) * 128, :])
                nc.sync.dma_start(xt[:, ND:], dqd[g * 128:(g + 1) * 128, :])
                xts.append(xt)
                pt = psT.tile([IN, 128], F32, tag="ptx")
                nc.tensor.transpose(pt[:], xt[:], ident[:])
                nc.scalar.activation(XT[:, g * 128:(g + 1) * 128], pt[:], AF.Copy)

            def mm(psum_ap, lhsT_parts, rhs_parts, Fr):
                nk = len(lhsT_parts)
                for c0 in range(0, Fr, CH):
                    ce = min(Fr, c0 + CH)
                    for ki in range(nk):
                        nc.tensor.matmul(psum_ap[:, c0:ce], lhsT_parts[ki],
                                         rhs_parts[ki][:, c0:ce],
                                         start=(ki == 0), stop=(ki == nk - 1))

            def fwd_layer(rhs_parts, Wparts, K, lidx):
                Zs, Ss = [], []
                nk = (K + 127) // 128
                for ot in range(KT):
                    ps = psB.tile([128, 1024], F32, tag="big")
                    lts = [Wparts[k][:, ot * 128:(ot + 1) * 128] for k in range(nk)]
                    mm(ps[:, 0:N], lts, rhs_parts, N)
                    Ap = scr.tile([128, N], F32, tag="Ap")
                    nc.scalar.activation(Ap[:], ps[0:128, 0:N], AF.Identity,
                                         bias=bs[lidx][ot][:])
                    ab = scr.tile([128, N], F32, tag="ab")
                    nc.scalar.activation(ab[:], Ap[:], AF.Abs)
                    ex = scr.tile([128, N], F32, tag="ex")
                    nc.scalar.activation(ex[:], ab[:], AF.Exp, scale=-1.0)
                    ln = scr.tile([128, N], F32, tag="ln")
                    nc.scalar.activation(ln[:], ex[:], AF.Ln, bias=1.0)
                    rl = scr.tile([128, N], F32, tag="rl")
                    nc.scalar.activation(rl[:], Ap[:], AF.Relu)
                    Z = actp.tile([128, N], F32R, tag=f"Zf{lidx % 2}_{ot}")
                    nc.vector.tensor_add(Z[:], rl[:], ln[:])
                    d = scr.tile([128, N], F32, tag="d")
                    nc.vector.tensor_sub(d[:], Ap[:], Z[:].bitcast(F32))
                    S = actp.tile([128, N], F32, tag=f"S{lidx}_{ot}")
                    nc.scalar.activation(S[:], d[:], AF.Exp)
                    Zs.append(Z)
                    Ss.append(S)
                return Zs, Ss

            Z1, S1 = fwd_layer([XT[:]], WT_r[0], IN, 0)
            Z2, S2 = fwd_layer([z[:] for z in Z1], WT_r[1], H, 1)
            Z3, S3 = fwd_layer([z[:] for z in Z2], WT_r[2], H, 2)
            _, S4 = fwd_layer([z[:] for z in Z3], WT_r[3], H, 3)

            D4, c4 = [], []
            for ot in range(KT):
                D = actp.tile([128, N], F32R, tag=f"Dr0_{ot}")
                nc.vector.tensor_scalar_mul(D[:], S4[ot][:], w4t[ot][:])
                D4.append(D)
                t1 = scr.tile([128, N], F32, tag="c4t")
                nc.vector.tensor_mul(t1[:], D[:].bitcast(F32), S4[ot][:])
                c = actp.tile([128, N], F32, tag=f"c4_{ot}")
                nc.vector.tensor_sub(c[:], D[:].bitcast(F32), t1[:])
                c4.append(c)

            def bwd_layer(Dup, Wparts, Sl, lidx, want_F):
                Ds, Fs = [], []
                for ot in range(KT):
                    ps = psB.tile([128, 1024], F32, tag="big")
                    lts = [Wparts[k][:, ot * 128:(ot + 1) * 128] for k in range(KT)]
                    mm(ps[:, 0:N], lts, [d[:] for d in Dup], N)
                    D = actp.tile([128, N], F32R, tag=f"Dr{lidx % 2}_{ot}")
                    nc.vector.tensor_mul(D[:], Sl[ot][:], ps[0:128, 0:N])
                    Ds.append(D)
                    if want_F:
                        F = actp.tile([128, N], F32, tag=f"F{lidx}_{ot}")
                        nc.vector.tensor_sub(F[:], ps[0:128, 0:N], D[:].bitcast(F32))
                        Fs.append(F)
                return Ds, Fs

            D3, F3 = bwd_layer(D4, Wn_r[3], S3, 3, True)
            D2, F2 = bwd_layer(D3, Wn_r[2], S2, 2, True)
            D1, _ = bwd_layer(D2, Wn_r[1], S1, 1, False)
            E1 = []
            for ot in range(KT):
                t1 = scr.tile([128, N], F32, tag="e1t")
                nc.vector.tensor_mul(t1[:], D1[ot][:].bitcast(F32), S1[ot][:])
                E = actp.tile([128, N], F32, tag=f"E1_{ot}")
                nc.vector.tensor_sub(E[:], D1[ot][:].bitcast(F32), t1[:])
                E1.append(E)

            Gps = psB.tile([128, 1024], F32, tag="big")
            mm(Gps[0:IN, 0:N], [W0_r[k][:] for k in range(KT)], [d[:] for d in D1], N)
            G = actp.tile([IN, N], F32)
            nc.scalar.activation(G[:], Gps[0:IN, 0:N], AF.Copy)

            aug = actp.tile([128, 13 * ND * NG], F32)
            aug4 = aug[:].rearrange("p (i j g) -> p i j g", i=ND, j=13, g=NG)

            for g in range(NG):
                ptg = psT.tile([128, IN], F32, tag="ptx")
                nc.tensor.transpose(ptg[0:128, 0:ND], G[0:ND, g * 128:(g + 1) * 128],
                                    ident[0:ND, 0:ND])
                nc.scalar.activation(aug4[:, :, 12, g], ptg[0:128, 0:ND], AF.Copy)

            Hc128 = CHq = None
            for b in range(NT):
                g, off = b // 2, (b % 2) * 64
                sl = slice(b * T, (b + 1) * T)

                def bca(Sten, ot):
                    return Sten[ot][:, sl].unsqueeze(1).broadcast_to((128, ND, T))

                def t3d(ps):
                    return ps[0:128, 0:FD].rearrange("p (d t) -> p d t", d=ND)

                Zd1 = []
                for ot in range(KT):
                    z = tgp.tile([128, ND, T], F32R, tag=f"ZdA_{ot}")
                    wq = W0q[ot][:].unsqueeze(2).broadcast_to((128, ND, T))
                    nc.vector.tensor_tensor(z[:], bca(S1, ot), wq, ALU.mult)
                    Zd1.append(z)

                def tang_fwd(Zin, Wparts, Sl, ztag):
                    outs = []
                    for ot in range(KT):
                        ps = psB.tile([128, 1024], F32, tag="big")
                        lts = [Wparts[k][:, ot * 128:(ot + 1) * 128] for k in range(KT)]
                        mm(ps[:, 0:FD], lts,
                           [z[:].rearrange("p d t -> p (d t)") for z in Zin], FD)
                        z = tgp.tile([128, ND, T], F32R, tag=f"Zd{ztag}_{ot}")
                        nc.vector.tensor_tensor(z[:], bca(Sl, ot), t3d(ps), ALU.mult)
                        outs.append(z)
                    return outs

                Zd2 = tang_fwd(Zd1, WT_r[1], S2, "B")
                Zd3 = tang_fwd(Zd2, WT_r[2], S3, "C")

                Dd4 = []
                for ot in range(KT):
                    ps = psB.tile([128, 1024], F32, tag="big")
                    lts = [WT_r[3][k][:, ot * 128:(ot + 1) * 128] for k in range(KT)]
                    mm(ps[:, 0:FD], lts,
                       [z[:].rearrange("p d t -> p (d t)") for z in Zd3], FD)
                    dd = tgp.tile([128, ND, T], F32R, tag=f"DdA_{ot}")
                    nc.vector.tensor_tensor(dd[:], bca(c4, ot), t3d(ps), ALU.mult)
                    Dd4.append(dd)

                def tang_bwd(Ddup, Wparts, Sl, Fl, Zdl, dtag):
                    outs = []
                    for ot in range(KT):
                        ps = psB.tile([128, 1024], F32, tag="big")
                        lts = [Wparts[k][:, ot * 128:(ot + 1) * 128] for k in range(KT)]
                        mm(ps[:, 0:FD], lts,
                           [d[:].rearrange("p d t -> p (d t)") for d in Ddup], FD)
                        tb = scr.tile([128, ND, T], F32, tag="tB")
                        nc.vector.tensor_tensor(tb[:], bca(Sl, ot), t3d(ps), ALU.mult)
                        ta = scr.tile([128, ND, T], F32, tag="tA2")
                        nc.vector.tensor_tensor(ta[:], bca(Fl, ot),
                                                Zdl[ot][:].bitcast(F32), ALU.mult)
                        dd = tgp.tile([128, ND, T], F32R, tag=f"Dd{dtag}_{ot}")
                        nc.gpsimd.tensor_add(dd[:].rearrange("p d t -> p (d t)"),
                                             ta[:].rearrange("p d t -> p (d t)"),
                                             tb[:].rearrange("p d t -> p (d t)"))
                        outs.append(dd)
                    return outs

                Dd3 = tang_bwd(Dd4, Wn_r[3], S3, F3, Zd3, "B")
                Dd2 = tang_bwd(Dd3, Wn_r[2], S2, F2, Zd2, "A")

                Dd1 = []
                for ot in range(KT):
                    ps = psB.tile([128, 1024], F32, tag="big")
                    lts = [Wn_r[1][k][:, ot * 128:(ot + 1) * 128] for k in range(KT)]
                    mm(ps[:, 0:FD], lts,
                       [d[:].rearrange("p d t -> p (d t)") for d in Dd2], FD)
                    tb = scr.tile([128, ND, T], F32, tag="tB")
                    nc.vector.tensor_tensor(tb[:], bca(S1, ot), t3d(ps), ALU.mult)
                    ta = scr.tile([128, ND, T], F32, tag="tA2")
                    wq = W0q[ot][:].unsqueeze(2).broadcast_to((128, ND, T))
                    nc.vector.tensor_tensor(ta[:], bca(E1, ot), wq, ALU.mult)
                    dd = tgp.tile([128, ND, T], F32R, tag=f"DdB_{ot}")
                    nc.gpsimd.tensor_add(dd[:].rearrange("p d t -> p (d t)"),
                                         ta[:].rearrange("p d t -> p (d t)"),
                                         tb[:].rearrange("p d t -> p (d t)"))
                    Dd1.append(dd)

                psH = psB.tile([128, 1024], F32, tag="big")
                mm(psH[0:IN, 0:FD], [W0_r[k][:] for k in range(KT)],
                   [d[:].rearrange("p d t -> p (d t)") for d in Dd1], FD)
                if off == 0:
                    Hc128 = scr.tile([IN, ND, 128], F32, tag="Hc")
                    CHq = scr.tile([128, ND * ND], F32, tag="CHq")
                nc.scalar.activation(Hc128[:, :, off:off + T],
                                     psH[0:IN, 0:FD].rearrange("p (d t) -> p d t", d=ND),
                                     AF.Copy)

                if off == 64:
                    for d in range(ND):
                        pt = psT.tile([128, IN], F32, tag="ptH")
                        nc.tensor.transpose(pt[:], Hc128[:, d, :], ident[0:IN, 0:IN])
                        nc.scalar.activation(CHq[:, d * ND:(d + 1) * ND],
                                             pt[:, 0:ND], AF.Copy)
                        nc.scalar.activation(aug4[:, :, d, g], pt[:, ND:IN], AF.Copy)
                    prod = scr.tile([128, ND, ND], F32, tag="prod")
                    qdv = xts[g][:, ND:IN].unsqueeze(1).broadcast_to((128, ND, ND))
                    nc.vector.tensor_tensor(prod[:], CHq[:].rearrange("p (i j) -> p i j", j=ND),
                                            qdv, ALU.mult)
                    cor = scr.tile([128, ND], F32, tag="cor")
                    nc.vector.tensor_reduce(cor[:].unsqueeze(2), prod[:], op=ALU.add,
                                            axis=mybir.AxisListType.X)
                    nc.vector.tensor_sub(aug4[:, :, 12, g], aug4[:, :, 12, g], cor[:])

            for i in range(ND):
                nc.vector.tensor_scalar_add(aug4[:, i, i, :], aug4[:, i, i, :], 0.01)

            for k in range(ND):
                piv = aug4[:, k, k, :]
                rec = scr.tile([128, NG], F32, tag="rec")
                nc.vector.reciprocal(rec[:], piv)
                nw = 12 - k
                rk = aug4[:, k, k + 1:13, :]
                recb = rec[:].unsqueeze(1).broadcast_to((128, nw, NG))
                nc.vector.scalar_tensor_tensor(rk, rk, -1.0, recb, ALU.mult, ALU.mult)
                for i in range(ND):
                    if i == k:
                        continue
                    fb = aug4[:, i, k, :].unsqueeze(1).broadcast_to((128, nw, NG))
                    tmv = scr.tile([128, nw, NG], F32, tag="gjt")
                    nc.vector.tensor_tensor(tmv[:], rk, fb, ALU.mult)
                    nc.vector.tensor_add(aug4[:, i, k + 1:13, :], aug4[:, i, k + 1:13, :], tmv[:])

            for g in range(NG):
                xo = scr.tile([128, ND], F32, tag="xo")
                nc.vector.tensor_scalar_mul(xo[:], aug4[:, :, 12, g], -1.0)
                nc.sync.dma_start(dout[g * 128:(g + 1) * 128, :], xo[:])

    nc.compile()
    return nc


def kernel(**inputs):
    q = np.ascontiguousarray(inputs["q"], dtype=np.float32)
    qdot = np.ascontiguousarray(inputs["qdot"], dtype=np.float32)
    if "nc" not in _cache:
        _cache["nc"] = build_kernel()
    nc = _cache["nc"]
    base = {
        "WT0": np.ascontiguousarray(inputs["W0"].T).astype(np.float32),
        "WT1": np.ascontiguousarray(inputs["W1"].T).astype(np.float32),
        "WT2": np.ascontiguousarray(inputs["W2"].T).astype(np.float32),
        "WT3": np.ascontiguousarray(inputs["W3"].T).astype(np.float32),
        "Wn1": np.ascontiguousarray(inputs["W1"]).astype(np.float32),
        "Wn2": np.ascontiguousarray(inputs["W2"]).astype(np.float32),
        "Wn3": np.ascontiguousarray(inputs["W3"]).astype(np.float32),
        "W0n": np.ascontiguousarray(inputs["W0"]).astype(np.float32),
        "b0": inputs["b0"].reshape(H, 1).astype(np.float32),
        "b1": inputs["b1"].reshape(H, 1).astype(np.float32),
        "b2": inputs["b2"].reshape(H, 1).astype(np.float32),
        "b3": inputs["b3"].reshape(H, 1).astype(np.float32),
        "w4": np.ascontiguousarray(inputs["W4"].reshape(H, 1)).astype(np.float32),
        "ident": np.eye(128, dtype=np.float32),
    }
    in_maps = []
    for c in range(NC):
        m = dict(base)
        m["q"] = q[c * N:(c + 1) * N]
        m["qdot"] = qdot[c * N:(c + 1) * N]
        in_maps.append(m)
    res = run_bass_kernel_spmd(nc, in_maps, core_ids=list(range(NC)))
    _cache["last_results"] = res
    out = np.concatenate([res.results[c]["qdd"] for c in range(NC)], axis=0)
    return out.astype(np.float32)



# revision 12
# speedup vs baseline: 1.5604x; 1.5604x over previous
# Lagrangian-NN qddot kernel for TRN2 (8 NeuronCores, data-parallel over batch).
#
# Math: scalar L(q,qdot) = MLP(24->256x4->1, softplus). Per sample:
#   M = d2L/dqdot2 + 0.01 I ; C = d2L/dqdot dq ; qddot = M^-1 (dL/dq - C qdot).
# fwd+bwd in f32r give the gradient; 12 qdot-direction forward-over-reverse
# tangents in bf16 give Hcols = H[:, 12:]; the 12x12 solve uses a 3-term
# Neumann series (M = 0.01(I + E), ||E|| < 0.04 for this data distribution).
# Tangent phase is processed layer-outer over quarter-batches so each weight
# block stays loaded across long 512-col matmul streams (keeps PE warm and
# minimizes LDWEIGHTS).
import sys
import numpy as np

for p in ("/opt/trn_rl_repo", "/root/.axon_site/_ro/trn_rl_repo"):
    if p not in sys.path:
        sys.path.insert(0, p)

import concourse.bass as bass
import concourse.mybir as mybir
import concourse.tile as tile
from concourse import bacc
from concourse.bass_utils import run_bass_kernel_spmd

F32 = mybir.dt.float32
F32R = mybir.dt.float32r
BF16 = mybir.dt.bfloat16
AF = mybir.ActivationFunctionType
ALU = mybir.AluOpType
AX = mybir.AxisListType

B, ND, H, NC = 8192, 12, 256, 8
N = B // NC          # 1024 samples per core
IN = 2 * ND          # 24
KT = H // 128        # 2 k-tiles
NG = N // 128        # 8 groups of 128 samples
NH = 4               # tangent quarter-batches
SQ = N // NH         # 256 samples per quarter
FQ = ND * SQ         # 3072 tangent free dim per quarter
GRP = 1024           # psum group: 2 chunks of 512 (2 banks)
NGRP = FQ // GRP     # 3 groups per (layer, quarter, ot)
DG = GRP // SQ       # 4 d-rows per group
CH = 512

_cache = {}


def build_kernel():
    nc = bacc.Bacc("TRN2", target_bir_lowering=False)
    dq = nc.dram_tensor("q", (N, ND), F32, kind="ExternalInput")
    dqd = nc.dram_tensor("qdot", (N, ND), F32, kind="ExternalInput")
    # f32 weights (bitcast to f32r at matmul): fwd lhsT = W^T, bwd lhsT = W
    dWT = [nc.dram_tensor(f"WT{l}", s, F32, kind="ExternalInput")
           for l, s in enumerate([(IN, H), (H, H), (H, H), (H, H)])]
    dWn = {l: nc.dram_tensor(f"Wn{l}", (H, H), F32, kind="ExternalInput") for l in (1, 2, 3)}
    dW0 = nc.dram_tensor("W0n", (H, IN), F32, kind="ExternalInput")
    # bf16 weights for the tangent phase
    dWTb = {l: nc.dram_tensor(f"WT{l}b", (H, H), BF16, kind="ExternalInput") for l in (1, 2, 3)}
    dWnb = {l: nc.dram_tensor(f"Wn{l}b", (H, H), BF16, kind="ExternalInput") for l in (1, 2, 3)}
    dW0b = nc.dram_tensor("W0nb", (H, IN), BF16, kind="ExternalInput")
    # W0[:, 12:] broadcast over samples, quarter-major: [H, NH*ND*SQ]
    dW0QX = nc.dram_tensor("W0QX", (H, NH * ND * SQ), BF16, kind="ExternalInput")
    dbs = [nc.dram_tensor(f"b{l}", (H, 1), F32, kind="ExternalInput") for l in range(4)]
    dw4 = nc.dram_tensor("w4", (H, 1), F32, kind="ExternalInput")
    dide = nc.dram_tensor("ident", (128, 128), F32, kind="ExternalInput")
    dout = nc.dram_tensor("qdd", (N, ND), F32, kind="ExternalOutput")

    with tile.TileContext(nc) as tc:
        with tc.tile_pool(name="wp", bufs=1) as wp, \
             tc.tile_pool(name="ap", bufs=1) as ap, \
             tc.tile_pool(name="psC", bufs=3, space="PSUM") as psC, \
             tc.tile_pool(name="psT", bufs=2, space="PSUM") as psT:

            ident = wp.tile([128, 128], F32)
            nc.sync.dma_start(ident[:], dide[:])

            def load_w(dram, P, Fr, tag, dt):
                parts = []
                for ki, p0 in enumerate(range(0, P, 128)):
                    pe = min(P, p0 + 128)
                    t = wp.tile([pe - p0, Fr], dt, tag=f"{tag}_{ki}")
                    nc.sync.dma_start(t[:], dram[p0:pe, :])
                    parts.append(t)
                return parts

            WTb = {l: load_w(dWTb[l], H, H, f"WT{l}b", BF16) for l in (1, 2, 3)}
            Wnb = {l: load_w(dWnb[l], H, H, f"Wn{l}b", BF16) for l in (1, 2, 3)}
            W0nb = load_w(dW0b, H, IN, "W0nb", BF16)
            bs = []
            for l in range(4):
                ps_ = []
                for ki in range(KT):
                    t = wp.tile([128, 1], F32, tag=f"b{l}_{ki}")
                    nc.sync.dma_start(t[:], dbs[l][ki * 128:(ki + 1) * 128, :])
                    ps_.append(t)
                bs.append(ps_)
            w4t = []
            for ki in range(KT):
                t = wp.tile([128, 1], F32, tag=f"w4_{ki}")
                nc.sync.dma_start(t[:], dw4[ki * 128:(ki + 1) * 128, :])
                w4t.append(t)

            # persistent activation-side tensors
            xall = ap.tile([128, NG, IN], F32)          # [s, g, (q,qdot)]
            for g in range(NG):
                nc.sync.dma_start(xall[:, g, 0:ND], dq[g * 128:(g + 1) * 128, :])
                nc.sync.dma_start(xall[:, g, ND:IN], dqd[g * 128:(g + 1) * 128, :])
            G = ap.tile([IN, N], F32)                   # gradient W0^T D1
            gq = ap.tile([128, NG, ND], F32)            # g_q transposed per sample
            Sb = {}                                     # bf16 aux for tangent
            for nm in ("S1b", "S2b", "S3b", "c4b", "F3b", "F2b", "E1b"):
                Sb[nm] = [ap.tile([128, N], BF16, tag=f"{nm}_{ot}", name=nm)
                          for ot in range(KT)]
            T2 = ap.tile([128, NG, IN, ND], F32)        # Hcols transposed: [s,g,r,d]

            def bcast(tiles, ot, h, nd):
                return tiles[ot][:, h * SQ:(h + 1) * SQ].unsqueeze(1) \
                    .broadcast_to((128, nd, SQ))

            # ---------------- forward + backward (f32r) ----------------
            with tc.tile_pool(name="fp", bufs=1) as fp:
                def load_round(dram, P, Fr, tag):
                    # f32r matmul operands must be written as f32r (verifier)
                    parts = []
                    for ki, p0 in enumerate(range(0, P, 128)):
                        pe = min(P, p0 + 128)
                        raw = fp.tile([pe - p0, Fr], F32, tag="wraw", bufs=2, name="wraw")
                        nc.sync.dma_start(raw[:], dram[p0:pe, :])
                        r = fp.tile([pe - p0, Fr], F32R, tag=f"{tag}_{ki}", name="wr")
                        nc.scalar.activation(r[:], raw[:], AF.Copy)
                        parts.append(r)
                    return parts

                WT = [load_round(dWT[l], (IN if l == 0 else H), H, f"WT{l}") for l in range(4)]
                Wn = {l: load_round(dWn[l], H, H, f"Wn{l}") for l in (1, 2, 3)}
                W0n = load_round(dW0, H, IN, "W0n")

                XT = fp.tile([IN, N], F32R)
                for g in range(NG):
                    pt = psT.tile([128, 128], F32, tag="pt")
                    nc.tensor.transpose(pt[0:IN, :], xall[:, g, :], ident[:])
                    nc.scalar.activation(XT[:, g * 128:(g + 1) * 128], pt[0:IN, :], AF.Copy)

                def mm_full(ps_ap, lhsT_parts, rhs_parts, Fr):
                    nk = len(lhsT_parts)
                    for c0 in range(0, Fr, CH):
                        ce = min(Fr, c0 + CH)
                        for ki in range(nk):
                            nc.tensor.matmul(ps_ap[:, c0:ce],
                                             lhsT_parts[ki],
                                             rhs_parts[ki][:, c0:ce],
                                             start=(ki == 0), stop=(ki == nk - 1))

                def fwd_layer(rhs_parts, Wparts, lidx, want_Z, sbname):
                    Zs, Ss = [], []
                    nk = len(Wparts)
                    for ot in range(KT):
                        ps = psC.tile([128, GRP], F32, tag="ch")
                        lts = [Wparts[k][:, ot * 128:(ot + 1) * 128] for k in range(nk)]
                        mm_full(ps[:, 0:N], lts, rhs_parts, N)
                        Ap_ = fp.tile([128, N], F32, tag="Ap")
                        nc.scalar.activation(Ap_[:], ps[0:128, 0:N], AF.Identity,
                                             bias=bs[lidx][ot][:])
                        ab = fp.tile([128, N], F32, tag="ab")
                        nc.scalar.activation(ab[:], Ap_[:], AF.Abs)
                        ex = fp.tile([128, N], F32, tag="ex")
                        nc.scalar.activation(ex[:], ab[:], AF.Exp, scale=-1.0)
                        ln = fp.tile([128, N], F32, tag="ln")
                        nc.scalar.activation(ln[:], ex[:], AF.Ln, bias=1.0)
                        Z = None
                        d = fp.tile([128, N], F32, tag="d")
                        if want_Z:
                            rl = fp.tile([128, N], F32, tag="rl")
                            nc.scalar.activation(rl[:], Ap_[:], AF.Relu)
                            Z = fp.tile([128, N], F32R, tag=f"Zf{lidx % 2}_{ot}")
                            nc.vector.tensor_add(Z[:], rl[:], ln[:])
                            nc.vector.tensor_sub(d[:], Ap_[:], Z[:].bitcast(F32))
                        else:
                            # S4 only: x - softplus(x) = -(relu(-x) + ln(1+e^-|x|))
                            rl = fp.tile([128, N], F32, tag="rl")
                            nc.scalar.activation(rl[:], Ap_[:], AF.Relu, scale=-1.0)
                            nc.vector.scalar_tensor_tensor(d[:], rl[:], -1.0, ln[:],
                                                           ALU.mult, ALU.subtract)
                        S = fp.tile([128, N], F32, tag=f"S{lidx}_{ot}")
                        nc.scalar.activation(S[:], d[:], AF.Exp)
                        if sbname is not None:
                            nc.gpsimd.tensor_copy(Sb[sbname][ot][:], S[:])
                        Zs.append(Z)
                        Ss.append(S)
                    return Zs, Ss

                Z1, S1 = fwd_layer([XT], WT[0], 0, True, "S1b")
                Z2, S2 = fwd_layer(Z1, WT[1], 1, True, "S2b")
                Z3, S3 = fwd_layer(Z2, WT[2], 2, True, "S3b")
                _, S4 = fwd_layer(Z3, WT[3], 3, False, None)

                # D4 = w4*S4 ; c4b = D4 - D4*S4 (bf16)
                D4 = []
                for ot in range(KT):
                    D = fp.tile([128, N], F32R, tag=f"Dr0_{ot}")
                    nc.vector.tensor_scalar_mul(D[:], S4[ot][:], w4t[ot][:])
                    D4.append(D)
                    t1 = fp.tile([128, N], F32, tag="c4t")
                    nc.vector.tensor_mul(t1[:], D[:].bitcast(F32), S4[ot][:])
                    nc.vector.tensor_sub(Sb["c4b"][ot][:], D[:].bitcast(F32), t1[:])

                def bwd_layer(Dup, Wparts, Sl, lidx, fbname):
                    Ds = []
                    for ot in range(KT):
                        ps = psC.tile([128, GRP], F32, tag="ch")
                        lts = [Wparts[k][:, ot * 128:(ot + 1) * 128] for k in range(KT)]
                        mm_full(ps[:, 0:N], lts, Dup, N)
                        D = fp.tile([128, N], F32R, tag=f"Dr{lidx % 2}_{ot}")
                        nc.vector.tensor_mul(D[:], Sl[ot][:], ps[0:128, 0:N])
                        Ds.append(D)
                        if fbname is not None:
                            nc.vector.tensor_sub(Sb[fbname][ot][:], ps[0:128, 0:N],
                                                 D[:].bitcast(F32))
                    return Ds

                D3 = bwd_layer(D4, Wn[3], S3, 3, "F3b")
                D2 = bwd_layer(D3, Wn[2], S2, 2, "F2b")
                D1 = bwd_layer(D2, Wn[1], S1, 1, None)
                for ot in range(KT):
                    t1 = fp.tile([128, N], F32, tag="c4t")
                    nc.vector.tensor_mul(t1[:], D1[ot][:].bitcast(F32), S1[ot][:])
                    nc.vector.tensor_sub(Sb["E1b"][ot][:], D1[ot][:].bitcast(F32), t1[:])

                # G = W0^T D1 ; gq[s,g,:] = G[0:12, :]^T
                psG = psC.tile([128, GRP], F32, tag="ch")
                mm_full(psG[0:IN, 0:N], W0n, D1, N)
                nc.scalar.activation(G[:], psG[0:IN, 0:N], AF.Copy)
                for g in range(NG):
                    pt = psT.tile([128, 128], F32, tag="pt")
                    nc.tensor.transpose(pt[:, 0:ND], G[0:ND, g * 128:(g + 1) * 128],
                                        ident[0:ND, 0:ND])
                    nc.scalar.activation(gq[:, g, :], pt[:, 0:ND], AF.Copy)

            # ---------------- tangent phase (bf16, quarter-batches) ----------------
            with tc.tile_pool(name="tp", bufs=1) as tp:
                for h in range(NH):
                    w0qx = []
                    for ki in range(KT):
                        t = tp.tile([128, ND, SQ], BF16, tag=f"w0qx_{ki}", name="w0qx")
                        nc.sync.dma_start(
                            t[:].rearrange("p d s -> p (d s)"),
                            dW0QX[ki * 128:(ki + 1) * 128, h * FQ:(h + 1) * FQ])
                        w0qx.append(t)

                    # Zd1 = S1b (bcast) * W0QX
                    Zd1 = []
                    for ot in range(KT):
                        z = tp.tile([128, ND, SQ], BF16, tag=f"ZdA_{ot}", name="Zd1")
                        nc.vector.tensor_tensor(z[:], bcast(Sb["S1b"], ot, h, ND),
                                                w0qx[ot][:], ALU.mult)
                        Zd1.append(z)

                    def tang_mm_groups(Wparts, rhs, ot):
                        # 3 psum groups of 1024, k-outer so each weight block
                        # streams 6 consecutive chunks
                        pss = [psC.tile([128, GRP], F32, tag="ch", name="psg")
                               for _ in range(NGRP)]
                        rfs = [r[:].rearrange("p d s -> p (d s)") for r in rhs]
                        for ki in range(KT):
                            lt = Wparts[ki][:, ot * 128:(ot + 1) * 128]
                            for grp in range(NGRP):
                                for c in range(GRP // CH):
                                    c0 = grp * GRP + c * CH
                                    nc.tensor.matmul(pss[grp][:, c * CH:(c + 1) * CH],
                                                     lt, rfs[ki][:, c0:c0 + CH],
                                                     start=(ki == 0), stop=(ki == KT - 1))
                        return pss

                    def psview(ps):
                        return ps[:, 0:GRP].rearrange("p (d s) -> p d s", s=SQ)

                    def gslice(t, grp):
                        return t[:, DG * grp:DG * (grp + 1), :]

                    def gbcast(tiles, ot, grp):
                        return tiles[ot][:, h * SQ:(h + 1) * SQ].unsqueeze(1) \
                            .broadcast_to((128, DG, SQ))

                    def tang_fwd(Zin, Wb, sbname, ztag):
                        outs = []
                        for ot in range(KT):
                            z = tp.tile([128, ND, SQ], BF16, tag=f"{ztag}_{ot}", name="Zd")
                            Ad = tp.tile([128, ND, SQ], BF16, tag=f"Ad_{ot}", name="Ad")
                            pss = tang_mm_groups(Wb, Zin, ot)
                            for grp in range(NGRP):
                                nc.scalar.activation(gslice(Ad, grp), psview(pss[grp]),
                                                     AF.Copy)
                            nc.vector.tensor_tensor(z[:], bcast(Sb[sbname], ot, h, ND),
                                                    Ad[:], ALU.mult)
                            outs.append(z)
                        return outs

                    Zd2 = tang_fwd(Zd1, WTb[1], "S2b", "ZdB")
                    Zd3 = tang_fwd(Zd2, WTb[2], "S3b", "ZdA")
                    Dd4 = tang_fwd(Zd3, WTb[3], "c4b", "DdA")

                    def tang_bwd(Ddup, Wb, sbname, Pin, ptiles, dtag):
                        outs = []
                        for ot in range(KT):
                            dd = tp.tile([128, ND, SQ], BF16, tag=f"{dtag}_{ot}", name="Dd")
                            # P = F_bcast * Zd  (all-bf16, SBUF-only -> gpsimd)
                            P = tp.tile([128, ND, SQ], BF16, tag=f"P_{ot}", name="P")
                            nc.gpsimd.tensor_tensor(P[:], bcast(Sb[Pin], ot, h, ND),
                                                    ptiles[ot][:], ALU.mult)
                            Yb = tp.tile([128, ND, SQ], BF16, tag=f"Ad_{ot}", name="Yb")
                            pss = tang_mm_groups(Wb, Ddup, ot)
                            for grp in range(NGRP):
                                nc.scalar.activation(gslice(Yb, grp), psview(pss[grp]),
                                                     AF.Copy)
                            nc.vector.tensor_tensor(dd[:], bcast(Sb[sbname], ot, h, ND),
                                                    Yb[:], ALU.mult)
                            nc.vector.tensor_tensor(dd[:], dd[:], P[:], ALU.add)
                            outs.append(dd)
                        return outs

                    Dd3 = tang_bwd(Dd4, Wnb[3], "S3b", "F3b", Zd3, "DdB")
                    Dd2 = tang_bwd(Dd3, Wnb[2], "S2b", "F2b", Zd2, "DdA")
                    Dd1 = tang_bwd(Dd2, Wnb[1], "S1b", "E1b", w0qx, "DdB")

                    # projection: T2[s, g, r, d] = sum_z Dd1[z,d,s] * W0n[z,r]
                    for gg in range(SQ // 128):
                        g = h * (SQ // 128) + gg
                        for d in range(ND):
                            pp = psT.tile([128, 128], F32, tag="pt", name="pp")
                            for ki in range(KT):
                                nc.tensor.matmul(
                                    pp[:, 0:IN],
                                    Dd1[ki][:, d, gg * 128:(gg + 1) * 128],
                                    W0nb[ki][:, 0:IN],
                                    start=(ki == 0), stop=(ki == KT - 1))
                            nc.scalar.activation(T2[:, g, :, d], pp[:, 0:IN], AF.Copy)

            # ---------------- solve: Neumann series ----------------
            # cor[s,g,d] = sum_j T2[s,g,j,d] * qdot[s,g,j]
            qdv = xall[:, :, ND:IN].unsqueeze(3).broadcast_to((128, NG, ND, ND))
            Pc = ap.tile([128, NG, ND, ND], F32, tag="Pc")
            nc.vector.tensor_tensor(Pc[:], T2[:, :, 0:ND, :], qdv, ALU.mult)
            cor = ap.tile([128, NG, ND], F32, tag="cor")
            nc.vector.tensor_reduce(cor[:].unsqueeze(3),
                                    Pc[:].rearrange("p g j d -> p g d j"),
                                    op=ALU.add, axis=AX.X)
            rhs = ap.tile([128, NG, ND], F32, tag="rhs")
            nc.vector.tensor_sub(rhs[:], gq[:], cor[:])

            Hq = T2[:, :, ND:IN, :]
            xprev = rhs
            for it in range(3):
                prod = ap.tile([128, NG, ND, ND], F32, tag="Pc", name="prod")
                xb = xprev[:].unsqueeze(2).broadcast_to((128, NG, ND, ND))
                nc.vector.tensor_tensor(prod[:], Hq, xb, ALU.mult)
                y = ap.tile([128, NG, ND], F32, tag=f"y{it % 2}", name="y")
                nc.vector.tensor_reduce(y[:].unsqueeze(3), prod[:], op=ALU.add, axis=AX.X)
                xn = ap.tile([128, NG, ND], F32, tag=f"x{it % 2}", name="xn")
                nc.vector.scalar_tensor_tensor(xn[:], y[:], -100.0, rhs[:],
                                               ALU.mult, ALU.add)
                xprev = xn

            o = ap.tile([128, NG, ND], F32, tag="o")
            nc.scalar.mul(o[:], xprev[:], 100.0)
            for g in range(NG):
                nc.sync.dma_start(dout[g * 128:(g + 1) * 128, :], o[:, g, :])

    nc.compile()
    return nc


def kernel(**inputs):
    import ml_dtypes
    q = np.ascontiguousarray(inputs["q"], dtype=np.float32)
    qdot = np.ascontiguousarray(inputs["qdot"], dtype=np.float32)
    if "nc" not in _cache:
        _cache["nc"] = build_kernel()
    nc = _cache["nc"]
    W = {l: inputs[f"W{l}"].astype(np.float32) for l in range(5)}
    bf = lambda a: np.ascontiguousarray(a).astype(ml_dtypes.bfloat16)
    W0b = bf(W[0])
    w0qx = np.ascontiguousarray(
        np.broadcast_to(np.asarray(W0b[:, ND:])[:, None, :, None],
                        (H, NH, ND, SQ)).reshape(H, NH * ND * SQ))
    base = {
        "WT0": np.ascontiguousarray(W[0].T),
        "WT1": np.ascontiguousarray(W[1].T),
        "WT2": np.ascontiguousarray(W[2].T),
        "WT3": np.ascontiguousarray(W[3].T),
        "Wn1": np.ascontiguousarray(W[1]),
        "Wn2": np.ascontiguousarray(W[2]),
        "Wn3": np.ascontiguousarray(W[3]),
        "W0n": np.ascontiguousarray(W[0]),
        "WT1b": bf(W[1].T), "WT2b": bf(W[2].T), "WT3b": bf(W[3].T),
        "Wn1b": bf(W[1]), "Wn2b": bf(W[2]), "Wn3b": bf(W[3]),
        "W0nb": W0b,
        "W0QX": w0qx,
        "b0": inputs["b0"].reshape(H, 1).astype(np.float32),
        "b1": inputs["b1"].reshape(H, 1).astype(np.float32),
        "b2": inputs["b2"].reshape(H, 1).astype(np.float32),
        "b3": inputs["b3"].reshape(H, 1).astype(np.float32),
        "w4": np.ascontiguousarray(W[4].reshape(H, 1)).astype(np.float32),
        "ident": np.eye(128, dtype=np.float32),
    }
    in_maps = []
    for c in range(NC):
        m = dict(base)
        m["q"] = q[c * N:(c + 1) * N]
        m["qdot"] = qdot[c * N:(c + 1) * N]
        in_maps.append(m)
    res = run_bass_kernel_spmd(nc, in_maps, core_ids=list(range(NC)))
    _cache["last_results"] = res
    out = np.concatenate([res.results[c]["qdd"] for c in range(NC)], axis=0)
    return out.astype(np.float32)


# revision 19
# speedup vs baseline: 1.7042x; 1.0921x over previous
# Lagrangian-NN qddot kernel for TRN2 (8 NeuronCores, data-parallel over batch).
#
# Math: scalar L(q,qdot) = MLP(24->256x4->1, softplus). Per sample:
#   M = d2L/dqdot2 + 0.01 I ; C = d2L/dqdot dq ; qddot = M^-1 (dL/dq - C qdot).
# fwd+bwd in f32r give the gradient; 12 qdot-direction forward-over-reverse
# tangents in bf16 give Hcols = H[:, 12:]; the 12x12 solve uses a 3-term
# Neumann series (M = 0.01(I + E), ||E|| < 0.04 for this data distribution).
# Tangent phase is processed layer-outer over quarter-batches so each weight
# block stays loaded across long 512-col matmul streams (keeps PE warm and
# minimizes LDWEIGHTS).
import sys
import numpy as np

for p in ("/opt/trn_rl_repo", "/root/.axon_site/_ro/trn_rl_repo"):
    if p not in sys.path:
        sys.path.insert(0, p)

import concourse.bass as bass
import concourse.mybir as mybir
import concourse.tile as tile
from concourse import bacc
from concourse.bass_utils import run_bass_kernel_spmd

F32 = mybir.dt.float32
F32R = mybir.dt.float32r
BF16 = mybir.dt.bfloat16
AF = mybir.ActivationFunctionType
ALU = mybir.AluOpType
AX = mybir.AxisListType

B, ND, H, NC = 8192, 12, 256, 8
N = B // NC          # 1024 samples per core
IN = 2 * ND          # 24
KT = H // 128        # 2 k-tiles
NG = N // 128        # 8 groups of 128 samples
NH = 4               # tangent quarter-batches
SQ = N // NH         # 256 samples per quarter
FQ = ND * SQ         # 3072 tangent free dim per quarter
GRP = 1024           # psum group: 2 chunks of 512 (2 banks)
NGRP = FQ // GRP     # 3 groups per (layer, quarter, ot)
DG = GRP // SQ       # 4 d-rows per group
CH = 512

_cache = {}


def build_kernel():
    nc = bacc.Bacc("TRN2", target_bir_lowering=False)
    dq = nc.dram_tensor("q", (N, ND), F32, kind="ExternalInput")
    dqd = nc.dram_tensor("qdot", (N, ND), F32, kind="ExternalInput")
    # f32 weights (bitcast to f32r at matmul): fwd lhsT = W^T, bwd lhsT = W
    dWT = [nc.dram_tensor(f"WT{l}", s, F32, kind="ExternalInput")
           for l, s in enumerate([(IN, H), (H, H), (H, H), (H, H)])]
    dWn = {l: nc.dram_tensor(f"Wn{l}", (H, H), F32, kind="ExternalInput") for l in (1, 2, 3)}
    dW0 = nc.dram_tensor("W0n", (H, IN), F32, kind="ExternalInput")
    # bf16 weights for the tangent phase
    dWTb = {l: nc.dram_tensor(f"WT{l}b", (H, H), BF16, kind="ExternalInput") for l in (1, 2, 3)}
    dWnb = {l: nc.dram_tensor(f"Wn{l}b", (H, H), BF16, kind="ExternalInput") for l in (1, 2, 3)}
    dW0b = nc.dram_tensor("W0nb", (H, IN), BF16, kind="ExternalInput")
    # W0[:, 12:] broadcast over samples, quarter-major: [H, NH*ND*SQ]
    dW0QX = nc.dram_tensor("W0QX", (H, NH * ND * SQ), BF16, kind="ExternalInput")
    dbs = [nc.dram_tensor(f"b{l}", (H, 1), F32, kind="ExternalInput") for l in range(4)]
    dnb3 = nc.dram_tensor("nb3", (H, 1), F32, kind="ExternalInput")
    dw4 = nc.dram_tensor("w4", (H, 1), F32, kind="ExternalInput")
    dide = nc.dram_tensor("ident", (128, 128), F32, kind="ExternalInput")
    dout = nc.dram_tensor("qdd", (N, ND), F32, kind="ExternalOutput")

    with tile.TileContext(nc) as tc:
        with tc.tile_pool(name="wp", bufs=1) as wp, \
             tc.tile_pool(name="ap", bufs=1) as ap, \
             tc.tile_pool(name="psC", bufs=3, space="PSUM") as psC, \
             tc.tile_pool(name="psT", bufs=2, space="PSUM") as psT:

            ident = wp.tile([128, 128], F32)
            nc.sync.dma_start(ident[:], dide[:])

            def load_w(dram, P, Fr, tag, dt):
                parts = []
                for ki, p0 in enumerate(range(0, P, 128)):
                    pe = min(P, p0 + 128)
                    t = wp.tile([pe - p0, Fr], dt, tag=f"{tag}_{ki}")
                    nc.sync.dma_start(t[:], dram[p0:pe, :])
                    parts.append(t)
                return parts

            WTb = {l: load_w(dWTb[l], H, H, f"WT{l}b", BF16) for l in (1, 2, 3)}
            Wnb = {l: load_w(dWnb[l], H, H, f"Wn{l}b", BF16) for l in (1, 2, 3)}
            W0nb = load_w(dW0b, H, IN, "W0nb", BF16)
            bs = []
            for l in range(4):
                ps_ = []
                for ki in range(KT):
                    t = wp.tile([128, 1], F32, tag=f"b{l}_{ki}")
                    nc.sync.dma_start(t[:], dbs[l][ki * 128:(ki + 1) * 128, :])
                    ps_.append(t)
                bs.append(ps_)
            w4t, nb3 = [], []
            for ki in range(KT):
                t = wp.tile([128, 1], F32, tag=f"w4_{ki}")
                nc.sync.dma_start(t[:], dw4[ki * 128:(ki + 1) * 128, :])
                w4t.append(t)
                t2_ = wp.tile([128, 1], F32, tag=f"nb3_{ki}", name="nb3t")
                nc.sync.dma_start(t2_[:], dnb3[ki * 128:(ki + 1) * 128, :])
                nb3.append(t2_)

            # persistent activation-side tensors
            xall = ap.tile([128, NG, IN], F32)          # [s, g, (q,qdot)]
            for g in range(NG):
                nc.sync.dma_start(xall[:, g, 0:ND], dq[g * 128:(g + 1) * 128, :])
                nc.sync.dma_start(xall[:, g, ND:IN], dqd[g * 128:(g + 1) * 128, :])
            G = ap.tile([IN, N], F32)                   # gradient W0^T D1
            gq = ap.tile([128, NG, ND], F32)            # g_q transposed per sample
            Sb = {}                                     # bf16 aux for tangent
            for nm in ("S1b", "S2b", "S3b", "c4b", "F3b", "F2b", "E1b"):
                Sb[nm] = [ap.tile([128, N], BF16, tag=f"{nm}_{ot}", name=nm)
                          for ot in range(KT)]
            T2 = ap.tile([128, NG, IN, ND], F32)        # Hcols transposed: [s,g,r,d]

            def bcast(tiles, ot, h, nd):
                return tiles[ot][:, h * SQ:(h + 1) * SQ].unsqueeze(1) \
                    .broadcast_to((128, nd, SQ))

            # ---------------- forward + backward (f32r) ----------------
            with tc.tile_pool(name="fp", bufs=1) as fp:
                def load_round(dram, P, Fr, tag):
                    # f32r matmul operands must be written as f32r (verifier)
                    parts = []
                    for ki, p0 in enumerate(range(0, P, 128)):
                        pe = min(P, p0 + 128)
                        raw = fp.tile([pe - p0, Fr], F32, tag="wraw", bufs=2, name="wraw")
                        nc.sync.dma_start(raw[:], dram[p0:pe, :])
                        r = fp.tile([pe - p0, Fr], F32R, tag=f"{tag}_{ki}", name="wr")
                        nc.scalar.activation(r[:], raw[:], AF.Copy)
                        parts.append(r)
                    return parts

                WT = [load_round(dWT[l], (IN if l == 0 else H), H, f"WT{l}") for l in range(4)]
                Wn = {l: load_round(dWn[l], H, H, f"Wn{l}") for l in (1, 2, 3)}
                W0n = load_round(dW0, H, IN, "W0n")

                XT = fp.tile([IN, N], F32R)
                for g in range(NG):
                    pt = psT.tile([128, 128], F32, tag="pt")
                    nc.tensor.transpose(pt[0:IN, :], xall[:, g, :], ident[:])
                    nc.scalar.activation(XT[:, g * 128:(g + 1) * 128], pt[0:IN, :], AF.Copy)

                def mm_full(ps_ap, lhsT_parts, rhs_parts, Fr):
                    nk = len(lhsT_parts)
                    for c0 in range(0, Fr, CH):
                        ce = min(Fr, c0 + CH)
                        for ki in range(nk):
                            nc.tensor.matmul(ps_ap[:, c0:ce],
                                             lhsT_parts[ki],
                                             rhs_parts[ki][:, c0:ce],
                                             start=(ki == 0), stop=(ki == nk - 1))

                def fwd_layer(rhs_parts, Wparts, lidx, want_Z, sbname):
                    Zs, Ss = [], []
                    nk = len(Wparts)
                    for ot in range(KT):
                        ps = psC.tile([128, GRP], F32, tag="ch")
                        lts = [Wparts[k][:, ot * 128:(ot + 1) * 128] for k in range(nk)]
                        mm_full(ps[:, 0:N], lts, rhs_parts, N)
                        psv = ps[0:128, 0:N]
                        b = bs[lidx][ot][:]
                        ab = fp.tile([128, N], F32, tag="ab")
                        nc.scalar.activation(ab[:], psv, AF.Abs, bias=b)
                        ex = fp.tile([128, N], F32, tag="ex")
                        nc.scalar.activation(ex[:], ab[:], AF.Exp, scale=-1.0)
                        ln = fp.tile([128, N], F32, tag="ln")
                        nc.scalar.activation(ln[:], ex[:], AF.Ln, bias=1.0)
                        Z = None
                        d = fp.tile([128, N], F32, tag="d")
                        if want_Z:
                            rl = fp.tile([128, N], F32, tag="rl")
                            nc.scalar.activation(rl[:], psv, AF.Relu, bias=b)
                            Z = fp.tile([128, N], F32R, tag=f"Zf{lidx % 2}_{ot}")
                            nc.vector.tensor_add(Z[:], rl[:], ln[:])
                            # d = (ps + b) - Z ; bias b applied in the S exp below
                            nc.vector.scalar_tensor_tensor(d[:], Z[:].bitcast(F32),
                                                           -1.0, psv, ALU.mult, ALU.add)
                            S = fp.tile([128, N], F32, tag=f"S{lidx}_{ot}")
                            nc.scalar.activation(S[:], d[:], AF.Exp, bias=b)
                        else:
                            # S4 only: x - softplus(x) = -(relu(-x-b) + ln(1+e^-|x+b|))
                            rl = fp.tile([128, N], F32, tag="rl")
                            nc.scalar.activation(rl[:], psv, AF.Relu, scale=-1.0,
                                                 bias=nb3[ot][:])
                            nc.vector.scalar_tensor_tensor(d[:], rl[:], -1.0, ln[:],
                                                           ALU.mult, ALU.subtract)
                            S = fp.tile([128, N], F32, tag=f"S{lidx}_{ot}")
                            nc.scalar.activation(S[:], d[:], AF.Exp)
                        if sbname is not None:
                            nc.gpsimd.tensor_copy(Sb[sbname][ot][:], S[:])
                        Zs.append(Z)
                        Ss.append(S)
                    return Zs, Ss

                Z1, S1 = fwd_layer([XT], WT[0], 0, True, "S1b")
                Z2, S2 = fwd_layer(Z1, WT[1], 1, True, "S2b")
                Z3, S3 = fwd_layer(Z2, WT[2], 2, True, "S3b")
                _, S4 = fwd_layer(Z3, WT[3], 3, False, None)

                # D4 = w4*S4 ; c4b = D4 - D4*S4 (bf16)
                D4 = []
                for ot in range(KT):
                    D = fp.tile([128, N], F32R, tag=f"Dr0_{ot}")
                    nc.vector.tensor_scalar_mul(D[:], S4[ot][:], w4t[ot][:])
                    D4.append(D)
                    t1 = fp.tile([128, N], F32, tag="c4t")
                    nc.vector.tensor_mul(t1[:], D[:].bitcast(F32), S4[ot][:])
                    nc.vector.tensor_sub(Sb["c4b"][ot][:], D[:].bitcast(F32), t1[:])

                def bwd_layer(Dup, Wparts, Sl, lidx, fbname):
                    Ds = []
                    for ot in range(KT):
                        ps = psC.tile([128, GRP], F32, tag="ch")
                        lts = [Wparts[k][:, ot * 128:(ot + 1) * 128] for k in range(KT)]
                        mm_full(ps[:, 0:N], lts, Dup, N)
                        D = fp.tile([128, N], F32R, tag=f"Dr{lidx % 2}_{ot}")
                        nc.vector.tensor_mul(D[:], Sl[ot][:], ps[0:128, 0:N])
                        Ds.append(D)
                        if fbname is not None:
                            nc.vector.tensor_sub(Sb[fbname][ot][:], ps[0:128, 0:N],
                                                 D[:].bitcast(F32))
                    return Ds

                D3 = bwd_layer(D4, Wn[3], S3, 3, "F3b")
                D2 = bwd_layer(D3, Wn[2], S2, 2, "F2b")
                D1 = bwd_layer(D2, Wn[1], S1, 1, None)
                for ot in range(KT):
                    t1 = fp.tile([128, N], F32, tag="c4t")
                    nc.vector.tensor_mul(t1[:], D1[ot][:].bitcast(F32), S1[ot][:])
                    nc.vector.tensor_sub(Sb["E1b"][ot][:], D1[ot][:].bitcast(F32), t1[:])

                # G = W0^T D1 ; gq[s,g,:] = G[0:12, :]^T
                psG = psC.tile([128, GRP], F32, tag="ch")
                mm_full(psG[0:IN, 0:N], W0n, D1, N)
                nc.scalar.activation(G[:], psG[0:IN, 0:N], AF.Copy)
                for g in range(NG):
                    pt = psT.tile([128, 128], F32, tag="pt")
                    nc.tensor.transpose(pt[:, 0:ND], G[0:ND, g * 128:(g + 1) * 128],
                                        ident[0:ND, 0:ND])
                    nc.scalar.activation(gq[:, g, :], pt[:, 0:ND], AF.Copy)

            # ---------------- tangent phase (bf16, quarter-batches) ----------------
            with tc.tile_pool(name="tp", bufs=1) as tp:
                for h in range(NH):
                    hp = h % 2  # parity-doubled buffers let quarter h+1 overlap h
                    w0qx = []
                    for ki in range(KT):
                        t = tp.tile([128, ND, SQ], BF16, tag=f"w0qx{hp}_{ki}", name="w0qx")
                        nc.sync.dma_start(
                            t[:].rearrange("p d s -> p (d s)"),
                            dW0QX[ki * 128:(ki + 1) * 128, h * FQ:(h + 1) * FQ])
                        w0qx.append(t)

                    # Zd1 = S1b (bcast) * W0QX
                    Zd1 = []
                    for ot in range(KT):
                        z = tp.tile([128, ND, SQ], BF16, tag=f"ZdA{hp}_{ot}", name="Zd1")
                        nc.vector.tensor_tensor(z[:], bcast(Sb["S1b"], ot, h, ND),
                                                w0qx[ot][:], ALU.mult)
                        Zd1.append(z)

                    def tang_mm_groups(Wparts, rhs, ot):
                        # 3 psum groups of 1024, k-outer so each weight block
                        # streams 6 consecutive chunks
                        pss = [psC.tile([128, GRP], F32, tag="ch", name="psg")
                               for _ in range(NGRP)]
                        rfs = [r[:].rearrange("p d s -> p (d s)") for r in rhs]
                        for ki in range(KT):
                            lt = Wparts[ki][:, ot * 128:(ot + 1) * 128]
                            for grp in range(NGRP):
                                for c in range(GRP // CH):
                                    c0 = grp * GRP + c * CH
                                    nc.tensor.matmul(pss[grp][:, c * CH:(c + 1) * CH],
                                                     lt, rfs[ki][:, c0:c0 + CH],
                                                     start=(ki == 0), stop=(ki == KT - 1))
                        return pss

                    def psview(ps):
                        return ps[:, 0:GRP].rearrange("p (d s) -> p d s", s=SQ)

                    def gslice(t, grp):
                        return t[:, DG * grp:DG * (grp + 1), :]

                    def gbcast(tiles, ot, grp):
                        return tiles[ot][:, h * SQ:(h + 1) * SQ].unsqueeze(1) \
                            .broadcast_to((128, DG, SQ))

                    def tang_fwd(Zin, Wb, sbname, ztag, direct):
                        outs = []
                        for ot in range(KT):
                            z = tp.tile([128, ND, SQ], BF16, tag=f"{ztag}{hp}_{ot}",
                                        name="Zd")
                            pss = tang_mm_groups(Wb, Zin, ot)
                            if direct:
                                for grp in range(NGRP):
                                    nc.vector.tensor_tensor(
                                        gslice(z, grp), gbcast(Sb[sbname], ot, grp),
                                        psview(pss[grp]), ALU.mult)
                            else:
                                Ad = tp.tile([128, ND, SQ], BF16, tag=f"Ad_{ot}",
                                             name="Ad")
                                for grp in range(NGRP):
                                    nc.scalar.activation(gslice(Ad, grp),
                                                         psview(pss[grp]), AF.Copy)
                                nc.vector.tensor_tensor(z[:],
                                                        bcast(Sb[sbname], ot, h, ND),
                                                        Ad[:], ALU.mult)
                            outs.append(z)
                        return outs

                    Zd2 = tang_fwd(Zd1, WTb[1], "S2b", "ZdB", False)
                    Zd3 = tang_fwd(Zd2, WTb[2], "S3b", "ZdA", True)
                    Dd4 = tang_fwd(Zd3, WTb[3], "c4b", "DdA", False)

                    def tang_bwd(Ddup, Wb, sbname, Pin, ptiles, dtag):
                        outs = []
                        for ot in range(KT):
                            dd = tp.tile([128, ND, SQ], BF16, tag=f"{dtag}{hp}_{ot}",
                                         name="Dd")
                            # P = F_bcast * Zd  (all-bf16, SBUF-only -> gpsimd)
                            P = tp.tile([128, ND, SQ], BF16, tag=f"P_{ot}", name="P")
                            nc.gpsimd.tensor_tensor(P[:], bcast(Sb[Pin], ot, h, ND),
                                                    ptiles[ot][:], ALU.mult)
                            Yb = tp.tile([128, ND, SQ], BF16, tag=f"Ad_{ot}",
                                         name="Yb")
                            pss = tang_mm_groups(Wb, Ddup, ot)
                            for grp in range(NGRP):
                                nc.scalar.activation(gslice(Yb, grp), psview(pss[grp]),
                                                     AF.Copy)
                            nc.vector.tensor_tensor(dd[:], bcast(Sb[sbname], ot, h, ND),
                                                    Yb[:], ALU.mult)
                            nc.vector.tensor_tensor(dd[:], dd[:], P[:], ALU.add)
                            outs.append(dd)
                        return outs

                    Dd3 = tang_bwd(Dd4, Wnb[3], "S3b", "F3b", Zd3, "DdB")
                    Dd2 = tang_bwd(Dd3, Wnb[2], "S2b", "F2b", Zd2, "DdA")
                    Dd1 = tang_bwd(Dd2, Wnb[1], "S1b", "E1b", w0qx, "DdB")

                    # projection: T2[s, g, r, d] = sum_z Dd1[z,d,s] * W0n[z,r]
                    for gg in range(SQ // 128):
                        g = h * (SQ // 128) + gg
                        for d in range(ND):
                            pp = psT.tile([128, 128], F32, tag="pt", name="pp")
                            for ki in range(KT):
                                nc.tensor.matmul(
                                    pp[:, 0:IN],
                                    Dd1[ki][:, d, gg * 128:(gg + 1) * 128],
                                    W0nb[ki][:, 0:IN],
                                    start=(ki == 0), stop=(ki == KT - 1))
                            nc.scalar.activation(T2[:, g, :, d], pp[:, 0:IN], AF.Copy)

            # ---------------- solve: Neumann series ----------------
            # cor[s,g,d] = sum_j T2[s,g,j,d] * qdot[s,g,j]
            qdv = xall[:, :, ND:IN].unsqueeze(3).broadcast_to((128, NG, ND, ND))
            Pc = ap.tile([128, NG, ND, ND], F32, tag="Pc")
            nc.vector.tensor_tensor(Pc[:], T2[:, :, 0:ND, :], qdv, ALU.mult)
            cor = ap.tile([128, NG, ND], F32, tag="cor")
            nc.vector.tensor_reduce(cor[:].unsqueeze(3),
                                    Pc[:].rearrange("p g j d -> p g d j"),
                                    op=ALU.add, axis=AX.X)
            rhs = ap.tile([128, NG, ND], F32, tag="rhs")
            nc.vector.tensor_sub(rhs[:], gq[:], cor[:])

            Hq = T2[:, :, ND:IN, :]
            xprev = rhs
            for it in range(3):
                prod = ap.tile([128, NG, ND, ND], F32, tag="Pc", name="prod")
                xb = xprev[:].unsqueeze(2).broadcast_to((128, NG, ND, ND))
                nc.vector.tensor_tensor(prod[:], Hq, xb, ALU.mult)
                y = ap.tile([128, NG, ND], F32, tag=f"y{it % 2}", name="y")
                nc.vector.tensor_reduce(y[:].unsqueeze(3), prod[:], op=ALU.add, axis=AX.X)
                xn = ap.tile([128, NG, ND], F32, tag=f"x{it % 2}", name="xn")
                nc.vector.scalar_tensor_tensor(xn[:], y[:], -100.0, rhs[:],
                                               ALU.mult, ALU.add)
                xprev = xn

            o = ap.tile([128, NG, ND], F32, tag="o")
            nc.scalar.mul(o[:], xprev[:], 100.0)
            for g in range(NG):
                nc.sync.dma_start(dout[g * 128:(g + 1) * 128, :], o[:, g, :])

    nc.compile()
    return nc


def kernel(**inputs):
    import ml_dtypes
    q = np.ascontiguousarray(inputs["q"], dtype=np.float32)
    qdot = np.ascontiguousarray(inputs["qdot"], dtype=np.float32)
    if "nc" not in _cache:
        _cache["nc"] = build_kernel()
    nc = _cache["nc"]
    W = {l: inputs[f"W{l}"].astype(np.float32) for l in range(5)}
    bf = lambda a: np.ascontiguousarray(a).astype(ml_dtypes.bfloat16)
    W0b = bf(W[0])
    w0qx = np.ascontiguousarray(
        np.broadcast_to(np.asarray(W0b[:, ND:])[:, None, :, None],
                        (H, NH, ND, SQ)).reshape(H, NH * ND * SQ))
    base = {
        "WT0": np.ascontiguousarray(W[0].T),
        "WT1": np.ascontiguousarray(W[1].T),
        "WT2": np.ascontiguousarray(W[2].T),
        "WT3": np.ascontiguousarray(W[3].T),
        "Wn1": np.ascontiguousarray(W[1]),
        "Wn2": np.ascontiguousarray(W[2]),
        "Wn3": np.ascontiguousarray(W[3]),
        "W0n": np.ascontiguousarray(W[0]),
        "WT1b": bf(W[1].T), "WT2b": bf(W[2].T), "WT3b": bf(W[3].T),
        "Wn1b": bf(W[1]), "Wn2b": bf(W[2]), "Wn3b": bf(W[3]),
        "W0nb": W0b,
        "W0QX": w0qx,
        "b0": inputs["b0"].reshape(H, 1).astype(np.float32),
        "b1": inputs["b1"].reshape(H, 1).astype(np.float32),
        "b2": inputs["b2"].reshape(H, 1).astype(np.float32),
        "b3": inputs["b3"].reshape(H, 1).astype(np.float32),
        "w4": np.ascontiguousarray(W[4].reshape(H, 1)).astype(np.float32),
        "nb3": np.ascontiguousarray(-inputs["b3"].reshape(H, 1)).astype(np.float32),
        "ident": np.eye(128, dtype=np.float32),
    }
    in_maps = []
    for c in range(NC):
        m = dict(base)
        m["q"] = q[c * N:(c + 1) * N]
        m["qdot"] = qdot[c * N:(c + 1) * N]
        in_maps.append(m)
    res = run_bass_kernel_spmd(nc, in_maps, core_ids=list(range(NC)))
    _cache["last_results"] = res
    out = np.concatenate([res.results[c]["qdd"] for c in range(NC)], axis=0)
    return out.astype(np.float32)


# revision 20
# speedup vs baseline: 1.9258x; 1.1301x over previous
# Lagrangian-NN qddot kernel for TRN2 (8 NeuronCores, data-parallel over batch).
#
# Math: scalar L(q,qdot) = MLP(24->256x4->1, softplus). Per sample:
#   M = d2L/dqdot2 + 0.01 I ; C = d2L/dqdot dq ; qddot = M^-1 (dL/dq - C qdot).
# fwd+bwd in f32r give the gradient; 12 qdot-direction forward-over-reverse
# tangents in bf16 give Hcols = H[:, 12:]; the 12x12 solve uses a 3-term
# Neumann series (M = 0.01(I + E), ||E|| < 0.04 for this data distribution).
# Tangent phase is processed layer-outer over quarter-batches so each weight
# block stays loaded across long 512-col matmul streams (keeps PE warm and
# minimizes LDWEIGHTS).
import sys
import numpy as np

for p in ("/opt/trn_rl_repo", "/root/.axon_site/_ro/trn_rl_repo"):
    if p not in sys.path:
        sys.path.insert(0, p)

import concourse.bass as bass
import concourse.mybir as mybir
import concourse.tile as tile
from concourse import bacc
from concourse.bass_utils import run_bass_kernel_spmd

F32 = mybir.dt.float32
F32R = mybir.dt.float32r
BF16 = mybir.dt.bfloat16
AF = mybir.ActivationFunctionType
ALU = mybir.AluOpType
AX = mybir.AxisListType

B, ND, H, NC = 8192, 12, 256, 8
N = B // NC          # 1024 samples per core
IN = 2 * ND          # 24
KT = H // 128        # 2 k-tiles
NG = N // 128        # 8 groups of 128 samples
NH = 4               # tangent quarter-batches
SQ = N // NH         # 256 samples per quarter
FQ = ND * SQ         # 3072 tangent free dim per quarter
GRP = 1024           # psum group: 2 chunks of 512 (2 banks)
NGRP = FQ // GRP     # 3 groups per (layer, quarter, ot)
DG = GRP // SQ       # 4 d-rows per group
CH = 512

_cache = {}


def build_kernel():
    nc = bacc.Bacc("TRN2", target_bir_lowering=False)
    dq = nc.dram_tensor("q", (N, ND), F32, kind="ExternalInput")
    dqd = nc.dram_tensor("qdot", (N, ND), F32, kind="ExternalInput")
    # f32 weights (bitcast to f32r at matmul): fwd lhsT = W^T, bwd lhsT = W
    dWT = [nc.dram_tensor(f"WT{l}", s, F32, kind="ExternalInput")
           for l, s in enumerate([(IN, H), (H, H), (H, H), (H, H)])]
    dWn = {l: nc.dram_tensor(f"Wn{l}", (H, H), F32, kind="ExternalInput") for l in (1, 2, 3)}
    dW0 = nc.dram_tensor("W0n", (H, IN), F32, kind="ExternalInput")
    # bf16 weights for the tangent phase
    dWTb = {l: nc.dram_tensor(f"WT{l}b", (H, H), BF16, kind="ExternalInput") for l in (1, 2, 3)}
    dWnb = {l: nc.dram_tensor(f"Wn{l}b", (H, H), BF16, kind="ExternalInput") for l in (1, 2, 3)}
    dW0b = nc.dram_tensor("W0nb", (H, IN), BF16, kind="ExternalInput")
    # W0[:, 12:] broadcast over samples, quarter-major: [H, NH*ND*SQ]
    dW0QX = nc.dram_tensor("W0QX", (H, NH * ND * SQ), BF16, kind="ExternalInput")
    dbs = [nc.dram_tensor(f"b{l}", (H, 1), F32, kind="ExternalInput") for l in range(4)]
    dnb3 = nc.dram_tensor("nb3", (H, 1), F32, kind="ExternalInput")
    dw4 = nc.dram_tensor("w4", (H, 1), F32, kind="ExternalInput")
    dide = nc.dram_tensor("ident", (128, 128), F32, kind="ExternalInput")
    dout = nc.dram_tensor("qdd", (N, ND), F32, kind="ExternalOutput")

    with tile.TileContext(nc) as tc:
        with tc.tile_pool(name="wp", bufs=1) as wp, \
             tc.tile_pool(name="ap", bufs=1) as ap, \
             tc.tile_pool(name="psC", bufs=3, space="PSUM") as psC, \
             tc.tile_pool(name="psT", bufs=2, space="PSUM") as psT:

            ident = wp.tile([128, 128], F32)
            nc.sync.dma_start(ident[:], dide[:])

            def load_w(dram, P, Fr, tag, dt):
                parts = []
                for ki, p0 in enumerate(range(0, P, 128)):
                    pe = min(P, p0 + 128)
                    t = wp.tile([pe - p0, Fr], dt, tag=f"{tag}_{ki}")
                    nc.sync.dma_start(t[:], dram[p0:pe, :])
                    parts.append(t)
                return parts

            WTb = {l: load_w(dWTb[l], H, H, f"WT{l}b", BF16) for l in (1, 2, 3)}
            Wnb = {l: load_w(dWnb[l], H, H, f"Wn{l}b", BF16) for l in (1, 2, 3)}
            W0nb = load_w(dW0b, H, IN, "W0nb", BF16)
            bs = []
            for l in range(4):
                ps_ = []
                for ki in range(KT):
                    t = wp.tile([128, 1], F32, tag=f"b{l}_{ki}")
                    nc.sync.dma_start(t[:], dbs[l][ki * 128:(ki + 1) * 128, :])
                    ps_.append(t)
                bs.append(ps_)
            w4t, nb3 = [], []
            for ki in range(KT):
                t = wp.tile([128, 1], F32, tag=f"w4_{ki}")
                nc.sync.dma_start(t[:], dw4[ki * 128:(ki + 1) * 128, :])
                w4t.append(t)
                t2_ = wp.tile([128, 1], F32, tag=f"nb3_{ki}", name="nb3t")
                nc.sync.dma_start(t2_[:], dnb3[ki * 128:(ki + 1) * 128, :])
                nb3.append(t2_)

            # persistent activation-side tensors
            xall = ap.tile([128, NG, IN], F32)          # [s, g, (q,qdot)]
            for g in range(NG):
                nc.sync.dma_start(xall[:, g, 0:ND], dq[g * 128:(g + 1) * 128, :])
                nc.sync.dma_start(xall[:, g, ND:IN], dqd[g * 128:(g + 1) * 128, :])
            G = ap.tile([IN, N], F32)                   # gradient W0^T D1
            gq = ap.tile([128, NG, ND], F32)            # g_q transposed per sample
            Sb = {}                                     # bf16 aux for tangent
            for nm in ("S1b", "S2b", "S3b", "c4b", "F3b", "F2b", "E1b"):
                Sb[nm] = [ap.tile([128, N], BF16, tag=f"{nm}_{ot}", name=nm)
                          for ot in range(KT)]
            T2 = ap.tile([128, NG, IN, ND], F32)        # Hcols transposed: [s,g,r,d]

            def bcast(tiles, ot, h, nd):
                return tiles[ot][:, h * SQ:(h + 1) * SQ].unsqueeze(1) \
                    .broadcast_to((128, nd, SQ))

            # ---------------- forward + backward (f32r) ----------------
            with tc.tile_pool(name="fp", bufs=1) as fp:
                def load_round(dram, P, Fr, tag):
                    # f32r matmul operands must be written as f32r (verifier)
                    parts = []
                    for ki, p0 in enumerate(range(0, P, 128)):
                        pe = min(P, p0 + 128)
                        raw = fp.tile([pe - p0, Fr], F32, tag="wraw", bufs=2, name="wraw")
                        nc.sync.dma_start(raw[:], dram[p0:pe, :])
                        r = fp.tile([pe - p0, Fr], F32R, tag=f"{tag}_{ki}", name="wr")
                        nc.scalar.activation(r[:], raw[:], AF.Copy)
                        parts.append(r)
                    return parts

                WT = [load_round(dWT[l], (IN if l == 0 else H), H, f"WT{l}") for l in range(4)]
                Wn = {l: load_round(dWn[l], H, H, f"Wn{l}") for l in (1, 2, 3)}
                W0n = load_round(dW0, H, IN, "W0n")

                XT = fp.tile([IN, N], F32R)
                for g in range(NG):
                    pt = psT.tile([128, 128], F32, tag="pt")
                    nc.tensor.transpose(pt[0:IN, :], xall[:, g, :], ident[:])
                    nc.scalar.activation(XT[:, g * 128:(g + 1) * 128], pt[0:IN, :], AF.Copy)

                def mm_full(ps_ap, lhsT_parts, rhs_parts, Fr):
                    nk = len(lhsT_parts)
                    for c0 in range(0, Fr, CH):
                        ce = min(Fr, c0 + CH)
                        for ki in range(nk):
                            nc.tensor.matmul(ps_ap[:, c0:ce],
                                             lhsT_parts[ki],
                                             rhs_parts[ki][:, c0:ce],
                                             start=(ki == 0), stop=(ki == nk - 1))

                def fwd_layer(rhs_parts, Wparts, lidx, want_Z, sbname):
                    Zs, Ss = [], []
                    nk = len(Wparts)
                    for ot in range(KT):
                        ps = psC.tile([128, GRP], F32, tag="ch")
                        lts = [Wparts[k][:, ot * 128:(ot + 1) * 128] for k in range(nk)]
                        mm_full(ps[:, 0:N], lts, rhs_parts, N)
                        psv = ps[0:128, 0:N]
                        b = bs[lidx][ot][:]
                        ab = fp.tile([128, N], F32, tag="ab")
                        nc.scalar.activation(ab[:], psv, AF.Abs, bias=b)
                        ex = fp.tile([128, N], F32, tag="ex")
                        nc.scalar.activation(ex[:], ab[:], AF.Exp, scale=-1.0)
                        ln = fp.tile([128, N], F32, tag="ln")
                        nc.scalar.activation(ln[:], ex[:], AF.Ln, bias=1.0)
                        Z = None
                        d = fp.tile([128, N], F32, tag="d")
                        if want_Z:
                            rl = fp.tile([128, N], F32, tag="rl")
                            nc.scalar.activation(rl[:], psv, AF.Relu, bias=b)
                            Z = fp.tile([128, N], F32R, tag=f"Zf{lidx % 2}_{ot}")
                            nc.vector.tensor_add(Z[:], rl[:], ln[:])
                            # d = (ps + b) - Z ; bias b applied in the S exp below
                            nc.vector.scalar_tensor_tensor(d[:], Z[:].bitcast(F32),
                                                           -1.0, psv, ALU.mult, ALU.add)
                            S = fp.tile([128, N], F32, tag=f"S{lidx}_{ot}")
                            nc.scalar.activation(S[:], d[:], AF.Exp, bias=b)
                        else:
                            # S4 only: x - softplus(x) = -(relu(-x-b) + ln(1+e^-|x+b|))
                            rl = fp.tile([128, N], F32, tag="rl")
                            nc.scalar.activation(rl[:], psv, AF.Relu, scale=-1.0,
                                                 bias=nb3[ot][:])
                            nc.vector.scalar_tensor_tensor(d[:], rl[:], -1.0, ln[:],
                                                           ALU.mult, ALU.subtract)
                            S = fp.tile([128, N], F32, tag=f"S{lidx}_{ot}")
                            nc.scalar.activation(S[:], d[:], AF.Exp)
                        if sbname is not None:
                            nc.vector.tensor_copy(Sb[sbname][ot][:], S[:])
                        Zs.append(Z)
                        Ss.append(S)
                    return Zs, Ss

                Z1, S1 = fwd_layer([XT], WT[0], 0, True, "S1b")
                Z2, S2 = fwd_layer(Z1, WT[1], 1, True, "S2b")
                Z3, S3 = fwd_layer(Z2, WT[2], 2, True, "S3b")
                _, S4 = fwd_layer(Z3, WT[3], 3, False, None)

                # D4 = w4*S4 ; c4b = D4 - D4*S4 (bf16)
                D4 = []
                for ot in range(KT):
                    D = fp.tile([128, N], F32R, tag=f"Dr0_{ot}")
                    nc.vector.tensor_scalar_mul(D[:], S4[ot][:], w4t[ot][:])
                    D4.append(D)
                    t1 = fp.tile([128, N], F32, tag="c4t")
                    nc.vector.tensor_mul(t1[:], D[:].bitcast(F32), S4[ot][:])
                    nc.vector.tensor_sub(Sb["c4b"][ot][:], D[:].bitcast(F32), t1[:])

                def bwd_layer(Dup, Wparts, Sl, lidx, fbname):
                    Ds = []
                    for ot in range(KT):
                        ps = psC.tile([128, GRP], F32, tag="ch")
                        lts = [Wparts[k][:, ot * 128:(ot + 1) * 128] for k in range(KT)]
                        mm_full(ps[:, 0:N], lts, Dup, N)
                        D = fp.tile([128, N], F32R, tag=f"Dr{lidx % 2}_{ot}")
                        nc.vector.tensor_mul(D[:], Sl[ot][:], ps[0:128, 0:N])
                        Ds.append(D)
                        if fbname is not None:
                            nc.vector.tensor_sub(Sb[fbname][ot][:], ps[0:128, 0:N],
                                                 D[:].bitcast(F32))
                    return Ds

                D3 = bwd_layer(D4, Wn[3], S3, 3, "F3b")
                D2 = bwd_layer(D3, Wn[2], S2, 2, "F2b")
                D1 = bwd_layer(D2, Wn[1], S1, 1, None)
                for ot in range(KT):
                    t1 = fp.tile([128, N], F32, tag="c4t")
                    nc.vector.tensor_mul(t1[:], D1[ot][:].bitcast(F32), S1[ot][:])
                    nc.vector.tensor_sub(Sb["E1b"][ot][:], D1[ot][:].bitcast(F32), t1[:])

                # G = W0^T D1 ; gq[s,g,:] = G[0:12, :]^T
                psG = psC.tile([128, GRP], F32, tag="ch")
                mm_full(psG[0:IN, 0:N], W0n, D1, N)
                nc.scalar.activation(G[:], psG[0:IN, 0:N], AF.Copy)
                for g in range(NG):
                    pt = psT.tile([128, 128], F32, tag="pt")
                    nc.tensor.transpose(pt[:, 0:ND], G[0:ND, g * 128:(g + 1) * 128],
                                        ident[0:ND, 0:ND])
                    nc.scalar.activation(gq[:, g, :], pt[:, 0:ND], AF.Copy)

            # ---------------- tangent phase (bf16, quarter-batches) ----------------
            with tc.tile_pool(name="tp", bufs=1) as tp:
                for h in range(NH):
                    hp = h % 2  # parity-doubled buffers let quarter h+1 overlap h
                    w0qx = []
                    for ki in range(KT):
                        t = tp.tile([128, ND, SQ], BF16, tag=f"w0qx{hp}_{ki}", name="w0qx")
                        nc.sync.dma_start(
                            t[:].rearrange("p d s -> p (d s)"),
                            dW0QX[ki * 128:(ki + 1) * 128, h * FQ:(h + 1) * FQ])
                        w0qx.append(t)

                    # Zd1 = S1b (bcast) * W0QX
                    Zd1 = []
                    for ot in range(KT):
                        z = tp.tile([128, ND, SQ], BF16, tag=f"ZdA{hp}_{ot}", name="Zd1")
                        nc.vector.tensor_tensor(z[:], bcast(Sb["S1b"], ot, h, ND),
                                                w0qx[ot][:], ALU.mult)
                        Zd1.append(z)

                    def tang_mm_groups(Wparts, rhs, ot):
                        # 3 psum groups of 1024, k-outer so each weight block
                        # streams 6 consecutive chunks
                        pss = [psC.tile([128, GRP], F32, tag="ch", name="psg")
                               for _ in range(NGRP)]
                        rfs = [r[:].rearrange("p d s -> p (d s)") for r in rhs]
                        for ki in range(KT):
                            lt = Wparts[ki][:, ot * 128:(ot + 1) * 128]
                            for grp in range(NGRP):
                                for c in range(GRP // CH):
                                    c0 = grp * GRP + c * CH
                                    nc.tensor.matmul(pss[grp][:, c * CH:(c + 1) * CH],
                                                     lt, rfs[ki][:, c0:c0 + CH],
                                                     start=(ki == 0), stop=(ki == KT - 1))
                        return pss

                    def psview(ps):
                        return ps[:, 0:GRP].rearrange("p (d s) -> p d s", s=SQ)

                    def gslice(t, grp):
                        return t[:, DG * grp:DG * (grp + 1), :]

                    def gbcast(tiles, ot, grp):
                        return tiles[ot][:, h * SQ:(h + 1) * SQ].unsqueeze(1) \
                            .broadcast_to((128, DG, SQ))

                    def tang_fwd(Zin, Wb, sbname, ztag, direct):
                        outs = []
                        for ot in range(KT):
                            z = tp.tile([128, ND, SQ], BF16, tag=f"{ztag}{hp}_{ot}",
                                        name="Zd")
                            pss = tang_mm_groups(Wb, Zin, ot)
                            if direct:
                                for grp in range(NGRP):
                                    nc.vector.tensor_tensor(
                                        gslice(z, grp), gbcast(Sb[sbname], ot, grp),
                                        psview(pss[grp]), ALU.mult)
                            else:
                                Ad = tp.tile([128, ND, SQ], BF16, tag=f"Ad_{ot}",
                                             name="Ad")
                                for grp in range(NGRP):
                                    nc.scalar.activation(gslice(Ad, grp),
                                                         psview(pss[grp]), AF.Copy)
                                nc.vector.tensor_tensor(z[:],
                                                        bcast(Sb[sbname], ot, h, ND),
                                                        Ad[:], ALU.mult)
                            outs.append(z)
                        return outs

                    Zd2 = tang_fwd(Zd1, WTb[1], "S2b", "ZdB", False)
                    Zd3 = tang_fwd(Zd2, WTb[2], "S3b", "ZdA", False)
                    Dd4 = tang_fwd(Zd3, WTb[3], "c4b", "DdA", False)

                    def tang_bwd(Ddup, Wb, sbname, Pin, ptiles, dtag):
                        outs = []
                        for ot in range(KT):
                            dd = tp.tile([128, ND, SQ], BF16, tag=f"{dtag}{hp}_{ot}",
                                         name="Dd")
                            # P = F_bcast * Zd  (all-bf16, SBUF-only -> gpsimd)
                            P = tp.tile([128, ND, SQ], BF16, tag=f"P_{ot}", name="P")
                            nc.vector.tensor_tensor(P[:], bcast(Sb[Pin], ot, h, ND),
                                                    ptiles[ot][:], ALU.mult)
                            Yb = tp.tile([128, ND, SQ], BF16, tag=f"Ad_{ot}",
                                         name="Yb")
                            pss = tang_mm_groups(Wb, Ddup, ot)
                            for grp in range(NGRP):
                                nc.scalar.activation(gslice(Yb, grp), psview(pss[grp]),
                                                     AF.Copy)
                            nc.vector.tensor_tensor(dd[:], bcast(Sb[sbname], ot, h, ND),
                                                    Yb[:], ALU.mult)
                            nc.vector.tensor_tensor(dd[:], dd[:], P[:], ALU.add)
                            outs.append(dd)
                        return outs

                    Dd3 = tang_bwd(Dd4, Wnb[3], "S3b", "F3b", Zd3, "DdB")
                    Dd2 = tang_bwd(Dd3, Wnb[2], "S2b", "F2b", Zd2, "DdA")
                    Dd1 = tang_bwd(Dd2, Wnb[1], "S1b", "E1b", w0qx, "DdB")

                    # projection: T2[s, g, r, d] = sum_z Dd1[z,d,s] * W0n[z,r]
                    for gg in range(SQ // 128):
                        g = h * (SQ // 128) + gg
                        for d in range(ND):
                            pp = psT.tile([128, 128], F32, tag="pt", name="pp")
                            for ki in range(KT):
                                nc.tensor.matmul(
                                    pp[:, 0:IN],
                                    Dd1[ki][:, d, gg * 128:(gg + 1) * 128],
                                    W0nb[ki][:, 0:IN],
                                    start=(ki == 0), stop=(ki == KT - 1))
                            nc.scalar.activation(T2[:, g, :, d], pp[:, 0:IN], AF.Copy)

            # ---------------- solve: Neumann series ----------------
            # cor[s,g,d] = sum_j T2[s,g,j,d] * qdot[s,g,j]
            qdv = xall[:, :, ND:IN].unsqueeze(3).broadcast_to((128, NG, ND, ND))
            Pc = ap.tile([128, NG, ND, ND], F32, tag="Pc")
            nc.vector.tensor_tensor(Pc[:], T2[:, :, 0:ND, :], qdv, ALU.mult)
            cor = ap.tile([128, NG, ND], F32, tag="cor")
            nc.vector.tensor_reduce(cor[:].unsqueeze(3),
                                    Pc[:].rearrange("p g j d -> p g d j"),
                                    op=ALU.add, axis=AX.X)
            rhs = ap.tile([128, NG, ND], F32, tag="rhs")
            nc.vector.tensor_sub(rhs[:], gq[:], cor[:])

            Hq = T2[:, :, ND:IN, :]
            xprev = rhs
            for it in range(3):
                prod = ap.tile([128, NG, ND, ND], F32, tag="Pc", name="prod")
                xb = xprev[:].unsqueeze(2).broadcast_to((128, NG, ND, ND))
                nc.vector.tensor_tensor(prod[:], Hq, xb, ALU.mult)
                y = ap.tile([128, NG, ND], F32, tag=f"y{it % 2}", name="y")
                nc.vector.tensor_reduce(y[:].unsqueeze(3), prod[:], op=ALU.add, axis=AX.X)
                xn = ap.tile([128, NG, ND], F32, tag=f"x{it % 2}", name="xn")
                nc.vector.scalar_tensor_tensor(xn[:], y[:], -100.0, rhs[:],
                                               ALU.mult, ALU.add)
                xprev = xn

            o = ap.tile([128, NG, ND], F32, tag="o")
            nc.scalar.mul(o[:], xprev[:], 100.0)
            for g in range(NG):
                nc.sync.dma_start(dout[g * 128:(g + 1) * 128, :], o[:, g, :])

    nc.compile()
    return nc


def kernel(**inputs):
    import ml_dtypes
    q = np.ascontiguousarray(inputs["q"], dtype=np.float32)
    qdot = np.ascontiguousarray(inputs["qdot"], dtype=np.float32)
    if "nc" not in _cache:
        _cache["nc"] = build_kernel()
    nc = _cache["nc"]
    W = {l: inputs[f"W{l}"].astype(np.float32) for l in range(5)}
    bf = lambda a: np.ascontiguousarray(a).astype(ml_dtypes.bfloat16)
    W0b = bf(W[0])
    w0qx = np.ascontiguousarray(
        np.broadcast_to(np.asarray(W0b[:, ND:])[:, None, :, None],
                        (H, NH, ND, SQ)).reshape(H, NH * ND * SQ))
    base = {
        "WT0": np.ascontiguousarray(W[0].T),
        "WT1": np.ascontiguousarray(W[1].T),
        "WT2": np.ascontiguousarray(W[2].T),
        "WT3": np.ascontiguousarray(W[3].T),
        "Wn1": np.ascontiguousarray(W[1]),
        "Wn2": np.ascontiguousarray(W[2]),
        "Wn3": np.ascontiguousarray(W[3]),
        "W0n": np.ascontiguousarray(W[0]),
        "WT1b": bf(W[1].T), "WT2b": bf(W[2].T), "WT3b": bf(W[3].T),
        "Wn1b": bf(W[1]), "Wn2b": bf(W[2]), "Wn3b": bf(W[3]),
        "W0nb": W0b,
        "W0QX": w0qx,
        "b0": inputs["b0"].reshape(H, 1).astype(np.float32),
        "b1": inputs["b1"].reshape(H, 1).astype(np.float32),
        "b2": inputs["b2"].reshape(H, 1).astype(np.float32),
        "b3": inputs["b3"].reshape(H, 1).astype(np.float32),
        "w4": np.ascontiguousarray(W[4].reshape(H, 1)).astype(np.float32),
        "nb3": np.ascontiguousarray(-inputs["b3"].reshape(H, 1)).astype(np.float32),
        "ident": np.eye(128, dtype=np.float32),
    }
    in_maps = []
    for c in range(NC):
        m = dict(base)
        m["q"] = q[c * N:(c + 1) * N]
        m["qdot"] = qdot[c * N:(c + 1) * N]
        in_maps.append(m)
    res = run_bass_kernel_spmd(nc, in_maps, core_ids=list(range(NC)))
    _cache["last_results"] = res
    out = np.concatenate([res.results[c]["qdd"] for c in range(NC)], axis=0)
    return out.astype(np.float32)
